# revision 1
# baseline (speedup 1.0000x reference)
"""Trainium2 Bass kernel for the sparse segment-softmax attention module.

Math: the reference computes, per nnz k,
    out[k] = segment_softmax((q1[b,i] + q2[b,j]) . v)  over segments (b, i).
Within a segment (fixed b, i), the q1[b,i].v term is constant and cancels in
softmax (shift invariance), as does the b2.v bias constant.  Hence
    out[k] = exp(u2[b, j_k]) / sum_{d in seg} exp(u2[b, j_d]),
    u2[b, n] = t2[b, n, :] . g,   g = W2^T v.

Device work per NeuronCore (2 batches of the 16, data-parallel over 8 cores):
  - stream t2 shard (4 MB; batch-0 tiles first, batch-1 backpressured via
    tile-pool reuse so batch 0 gets the full HBM ramp)
  - fused multiply+reduce on DVE+ACT -> u2acc [128, 4]
  - per-column: PE transpose, exp fused into the PSUM->SBUF copy, PE
    ones-outer-product to replicate the row across partitions -> table
  - gather exp(u2)[j] with the hardware pool-engine IndirectCopy in two
    1024-position halves (ISA dst limit); each position serves all 8 Q7
    groups in parallel.  The pool queue serializes a ~28ns/position
    post-drain per IC, so the last IC's drain is overlapped with the
    normalize/store tail.
  - compact the group-redundant gather output with one SBUF->SBUF DMA per
    half, windowed softmax normalize on DVE, store.
"""

import os
from contextlib import ExitStack

import numpy as np

B = 16
N1 = 512
N2 = 512
F2 = 1024
DEG = 32
NNZ = B * N1 * DEG
NCORES = 8
BPC = B // NCORES  # batches per core

_CACHE: dict = {}


def _build_program():
    import concourse.bacc as bacc
    import concourse.mybir as mybir
    import concourse.tile as tile

    fp32 = mybir.dt.float32
    bf16 = mybir.dt.bfloat16
    u16 = mybir.dt.uint16

    nc = bacc.Bacc("TRN2", target_bir_lowering=False, debug=False)

    t2s = nc.dram_tensor("t2s", [BPC, N2, F2], bf16, kind="ExternalInput")
    idxs = nc.dram_tensor("idxs", [BPC, 128, 128], u16, kind="ExternalInput")
    gbc = nc.dram_tensor("gbc", [128, F2], bf16, kind="ExternalInput")
    onesr = nc.dram_tensor("onesr", [1, 128], fp32, kind="ExternalInput")
    ident = nc.dram_tensor("ident", [128, 128], fp32, kind="ExternalInput")
    out = nc.dram_tensor("out", [BPC, 128, 128], fp32, kind="ExternalOutput")

    with tile.TileContext(nc) as tc, ExitStack() as ctx:
        constp = ctx.enter_context(tc.tile_pool(name="const", bufs=1))
        t2p = ctx.enter_context(tc.tile_pool(name="t2p", bufs=4))
        prodp = ctx.enter_context(tc.tile_pool(name="prodp", bufs=3))
        smallp = ctx.enter_context(tc.tile_pool(name="small", bufs=2))
        gathp = ctx.enter_context(tc.tile_pool(name="gath", bufs=2))
        psum_tp = ctx.enter_context(tc.tile_pool(name="pst", bufs=2, space="PSUM"))
        psum_rp = ctx.enter_context(tc.tile_pool(name="psr", bufs=2, space="PSUM"))

        # Small inputs on the scalar (ACT HWDGE) ring so the sync ring
        # belongs to the t2 stream; g first (it gates the mults).  The
        # smalls share SDMA bandwidth with the t2 stream and land late
        # (~25 us) but nothing they gate is on the critical path earlier
        # than that: the kernel is bound by the IndirectCopy issue rate.
        g_sb = constp.tile([128, F2], bf16)
        nc.scalar.dma_start(g_sb[:], gbc[:])
        ones_t = constp.tile([1, 128], fp32)
        nc.scalar.dma_start(ones_t[:], onesr[:])
        ident_t = constp.tile([128, 128], fp32)
        nc.scalar.dma_start(ident_t[:], ident[:])
        idx_tiles = []
        for b in range(BPC):
            idx_t = constp.tile([128, 128], u16, tag=f"idx{b}", name=f"idx_t{b}")
            nc.scalar.dma_start(idx_t[:], idxs[b])
            idx_tiles.append(idx_t)

        # t2 stream: bufs=4 on the t2 tag means batch 1's tile t reuses the
        # buffer of batch 0's tile t, so its DMA dispatches only once batch
        # 0's mult consumed it -> batch 0 gets the full HBM ramp.
        t2_tiles = []
        for b in range(BPC):
            for t in range(4):
                t2t = t2p.tile([128, F2], bf16, tag="t2", name=f"t2t_{b}_{t}")
                nc.sync.dma_start(t2t[:], t2s[b, 128 * t : 128 * (t + 1), :])
                t2_tiles.append(t2t)

        tables = []
        for b in range(BPC):
            # ---- u2 = t2[b] @ g, one column per 128-row tile.  The row
            # reduction is split: ACT accumulates the first half while DVE
            # reduces the second, halving the serial ACT chain on the
            # critical path to the first gather ----
            u2accA = smallp.tile([128, 4], fp32, tag="u2accA")
            u2accB = smallp.tile([128, 4], fp32, tag="u2accB")
            u2acc = smallp.tile([128, 4], fp32, tag="u2acc")
            psum_row = psum_rp.tile([1, 512], fp32, tag="prow")
            for t in range(4):
                t2t = t2_tiles[4 * b + t]
                prod = prodp.tile([128, F2], bf16, tag="prod")
                nc.vector.tensor_tensor(
                    out=prod[:], in0=t2t[:], in1=g_sb[:], op=mybir.AluOpType.mult
                )
                nc.scalar.activation(
                    prod[:, 0:512],
                    prod[:, 0:512],
                    func=mybir.ActivationFunctionType.Copy,
                    accum_out=u2accA[:, t : t + 1],
                )
                nc.vector.tensor_reduce(
                    out=u2accB[:, t : t + 1],
                    in_=prod[:, 512:1024],
                    axis=mybir.AxisListType.X,
                    op=mybir.AluOpType.add,
                )
                nc.vector.tensor_tensor(
                    out=u2acc[:, t : t + 1],
                    in0=u2accA[:, t : t + 1],
                    in1=u2accB[:, t : t + 1],
                    op=mybir.AluOpType.add,
                )
                # column t -> row slice [1, 128t:128t+128] via PE transpose
                nc.tensor.matmul(
                    psum_row[:, 128 * t : 128 * (t + 1)],
                    u2acc[:, t : t + 1],
                    ident_t[:],
                    is_transpose=True,
                )
            # exp fused into the single PSUM evacuation, then one
            # ones-outer-product replicates the row across all partitions
            row_all = smallp.tile([1, 512], fp32, tag="rowall")
            nc.scalar.activation(
                row_all[:], psum_row[:], func=mybir.ActivationFunctionType.Exp
            )
            psum_tab = psum_tp.tile([128, 512], fp32, tag="ptab")
            nc.tensor.matmul(
                psum_tab[:], ones_t[:], row_all[:], start=True, stop=True
            )
            table_b = gathp.tile([128, 512], fp32, tag=f"table{b}")
            nc.scalar.copy(table_b[:], psum_tab[:])
            tables.append(table_b)

        # ---- gathers: all four ICs back-to-back on the pool queue; each
        # IC's data is ready ~1.7us after dispatch, the ~28us post-drain
        # only blocks the next IC, and the last drain overlaps the tail ----
        gouts = []
        for b in range(BPC):
            gout = gathp.tile([128, 2048], fp32, tag=f"gout{b}")
            nc.gpsimd.indirect_copy(
                gout[:, 0:1024], tables[b][:], idx_tiles[b][:, 0:64], True
            )
            nc.gpsimd.indirect_copy(
                gout[:, 1024:2048], tables[b][:], idx_tiles[b][:, 64:128], True
            )
            gouts.append(gout)

        for b in range(BPC):
            gout = gouts[b]
            # ---- compact: one partition per 16-group holds the real data ----
            C = smallp.tile([128, 128], fp32, tag=f"C{b}")
            gsel = gout[:].rearrange("(g s) k -> g s k", s=16)[:, 0, :]
            nc.sync.dma_start(C[:], gsel)

            # ---- windowed softmax normalize (4 segments x 32 / partition) --
            C3 = C[:].rearrange("p (s d) -> p s d", d=32)
            S = smallp.tile([128, 4], fp32, tag="S")
            nc.vector.tensor_reduce(
                out=S[:], in_=C3, axis=mybir.AxisListType.X, op=mybir.AluOpType.add
            )
            R = smallp.tile([128, 4], fp32, tag="R")
            nc.vector.reciprocal(R[:], S[:])
            O = smallp.tile([128, 128], fp32, tag="O")
            O3 = O[:].rearrange("p (s d) -> p s d", d=32)
            R3 = R[:].unsqueeze(2).broadcast_to((128, 4, 32))
            nc.vector.tensor_tensor(
                out=O3, in0=C3, in1=R3, op=mybir.AluOpType.mult
            )

            nc.sync.dma_start(out[b], O[:])

    nc.compile()
    return nc


def _prep_core_inputs(t2, idx_j, W2, v):
    import ml_dtypes

    g = (W2.T.astype(np.float64) @ v.astype(np.float64)).astype(np.float32)
    gbc = np.ascontiguousarray(
        np.broadcast_to(g.reshape(1, F2), (128, F2)).astype(ml_dtypes.bfloat16)
    )
    t2 = t2.astype(ml_dtypes.bfloat16)
    onesr = np.ones((1, 128), dtype=np.float32)
    ident = np.eye(128, dtype=np.float32)

    j3 = np.ascontiguousarray(idx_j.reshape(B, N1, DEG).astype(np.uint16))
    in_maps = []
    for c in range(NCORES):
        bb = slice(BPC * c, BPC * (c + 1))
        t2s = np.ascontiguousarray(t2[bb])
        idxs = np.empty((BPC, 128, 128), dtype=np.uint16)
        for lb in range(BPC):
            gb = BPC * c + lb
            for grp in range(8):
                stream = j3[gb, 64 * grp : 64 * (grp + 1), :].reshape(2048)
                idxs[lb, 16 * grp : 16 * (grp + 1), :] = stream.reshape(128, 16).T
        in_maps.append(
            {
                "t2s": t2s,
                "idxs": idxs,
                "gbc": gbc,
                "onesr": onesr,
                "ident": ident,
            }
        )
    return in_maps


def kernel(t1, t2, idx_b, idx_i, idx_j, W1, b1, W2, b2, v):
    from concourse.bass_utils import run_bass_kernel_spmd

    if "nc" not in _CACHE:
        _CACHE["nc"] = _build_program()
    nc = _CACHE["nc"]

    in_maps = _prep_core_inputs(
        np.asarray(t2, dtype=np.float32),
        np.asarray(idx_j),
        np.asarray(W2, dtype=np.float32),
        np.asarray(v, dtype=np.float32),
    )
    trace = bool(int(os.environ.get("KERNEL_TRACE", "0")))
    last_err = None
    for _attempt in range(3):
        try:
            res = run_bass_kernel_spmd(nc, in_maps, list(range(NCORES)), trace=trace)
            break
        except Exception as e:  # transient NRT_EXEC_UNIT_UNRECOVERABLE wedges
            last_err = e
    else:
        raise last_err
    _CACHE["last_results"] = res
    outs = [r["out"].reshape(BPC * N1 * DEG) for r in res.results]
    return np.concatenate(outs).astype(np.float32)



# revision 8
# speedup vs baseline: 1.0037x; 1.0037x over previous
"""Trainium2 Bass kernel for the sparse segment-softmax attention module.

Math: the reference computes, per nnz k,
    out[k] = segment_softmax((q1[b,i] + q2[b,j]) . v)  over segments (b, i).
Within a segment (fixed b, i), the q1[b,i].v term is constant and cancels in
softmax (shift invariance), as does the b2.v bias constant.  Hence
    out[k] = exp(u2[b, j_k]) / sum_{d in seg} exp(u2[b, j_d]),
    u2[b, n] = t2[b, n, :] . g,   g = W2^T v.

Device work per NeuronCore (2 batches of the 16, data-parallel over 8 cores):
  - stream t2 shard (4 MB; batch-0 tiles first, batch-1 backpressured via
    tile-pool reuse so batch 0 gets the full HBM ramp)
  - fused multiply+reduce on DVE+ACT -> u2acc [128, 4]
  - per-column: PE transpose, exp fused into the PSUM->SBUF copy, PE
    ones-outer-product to replicate the row across partitions -> table
  - gather exp(u2)[j] with the hardware pool-engine IndirectCopy in two
    1024-position halves (ISA dst limit); each position serves all 8 Q7
    groups in parallel.  The pool queue serializes a ~28ns/position
    post-drain per IC, so the last IC's drain is overlapped with the
    normalize/store tail.
  - compact the group-redundant gather output with one SBUF->SBUF DMA per
    half, windowed softmax normalize on DVE, store.
"""

import os
from contextlib import ExitStack

import numpy as np

B = 16
N1 = 512
N2 = 512
F2 = 1024
DEG = 32
NNZ = B * N1 * DEG
NCORES = 8
BPC = B // NCORES  # batches per core

_CACHE: dict = {}


def _build_program():
    import concourse.bacc as bacc
    import concourse.mybir as mybir
    import concourse.tile as tile

    fp32 = mybir.dt.float32
    bf16 = mybir.dt.bfloat16
    i16 = mybir.dt.int16

    nc = bacc.Bacc("TRN2", target_bir_lowering=False, debug=False)

    t2s = nc.dram_tensor("t2s", [BPC, N2, F2], bf16, kind="ExternalInput")
    idxs = nc.dram_tensor("idxs", [BPC, 128, 128], i16, kind="ExternalInput")
    gbc = nc.dram_tensor("gbc", [128, F2], bf16, kind="ExternalInput")
    onesr = nc.dram_tensor("onesr", [1, 128], fp32, kind="ExternalInput")
    ident = nc.dram_tensor("ident", [128, 128], fp32, kind="ExternalInput")
    out = nc.dram_tensor("out", [BPC, 128, 128], fp32, kind="ExternalOutput")

    with tile.TileContext(nc) as tc, ExitStack() as ctx:
        constp = ctx.enter_context(tc.tile_pool(name="const", bufs=1))
        t2p = ctx.enter_context(tc.tile_pool(name="t2p", bufs=4))
        prodp = ctx.enter_context(tc.tile_pool(name="prodp", bufs=3))
        smallp = ctx.enter_context(tc.tile_pool(name="small", bufs=2))
        gathp = ctx.enter_context(tc.tile_pool(name="gath", bufs=2))
        psum_tp = ctx.enter_context(tc.tile_pool(name="pst", bufs=2, space="PSUM"))
        psum_rp = ctx.enter_context(tc.tile_pool(name="psr", bufs=2, space="PSUM"))

        # Small inputs on the scalar (ACT HWDGE) ring so the sync ring
        # belongs to the t2 stream; g first (it gates the mults).  The
        # smalls share SDMA bandwidth with the t2 stream and land late
        # (~25 us) but nothing they gate is on the critical path earlier
        # than that: the kernel is bound by the IndirectCopy issue rate.
        g_sb = constp.tile([128, F2], bf16)
        nc.scalar.dma_start(g_sb[:], gbc[:])
        ones_t = constp.tile([1, 128], fp32)
        nc.scalar.dma_start(ones_t[:], onesr[:])
        ident_t = constp.tile([128, 128], fp32)
        nc.scalar.dma_start(ident_t[:], ident[:])
        idx_tiles = []
        for b in range(BPC):
            idx_t = constp.tile([128, 128], i16, tag=f"idx{b}", name=f"idx_t{b}")
            nc.scalar.dma_start(idx_t[:], idxs[b])
            idx_tiles.append(idx_t)

        # t2 stream: bufs=4 on the t2 tag means batch 1's tile t reuses the
        # buffer of batch 0's tile t, so its DMA dispatches only once batch
        # 0's mult consumed it -> batch 0 gets the full HBM ramp.
        t2_tiles = []
        for b in range(BPC):
            for t in range(4):
                t2t = t2p.tile([128, F2], bf16, tag="t2", name=f"t2t_{b}_{t}")
                nc.sync.dma_start(t2t[:], t2s[b, 128 * t : 128 * (t + 1), :])
                t2_tiles.append(t2t)

        tables = []
        for b in range(BPC):
            # ---- u2 = t2[b] @ g, one column per 128-row tile.  The row
            # reduction is split: ACT accumulates the first half while DVE
            # reduces the second, halving the serial ACT chain on the
            # critical path to the first gather ----
            u2accA = smallp.tile([128, 4], fp32, tag="u2accA")
            u2accB = smallp.tile([128, 4], fp32, tag="u2accB")
            u2acc = smallp.tile([128, 4], fp32, tag="u2acc")
            psum_row = psum_rp.tile([1, 512], fp32, tag="prow")
            for t in range(4):
                t2t = t2_tiles[4 * b + t]
                prod = prodp.tile([128, F2], bf16, tag="prod")
                nc.vector.tensor_tensor(
                    out=prod[:], in0=t2t[:], in1=g_sb[:], op=mybir.AluOpType.mult
                )
                nc.scalar.activation(
                    prod[:, 0:512],
                    prod[:, 0:512],
                    func=mybir.ActivationFunctionType.Copy,
                    accum_out=u2accA[:, t : t + 1],
                )
                nc.vector.tensor_reduce(
                    out=u2accB[:, t : t + 1],
                    in_=prod[:, 512:1024],
                    axis=mybir.AxisListType.X,
                    op=mybir.AluOpType.add,
                )
                nc.vector.tensor_tensor(
                    out=u2acc[:, t : t + 1],
                    in0=u2accA[:, t : t + 1],
                    in1=u2accB[:, t : t + 1],
                    op=mybir.AluOpType.add,
                )
                # column t -> row slice [1, 128t:128t+128] via PE transpose
                nc.tensor.matmul(
                    psum_row[:, 128 * t : 128 * (t + 1)],
                    u2acc[:, t : t + 1],
                    ident_t[:],
                    is_transpose=True,
                )
            # exp fused into the single PSUM evacuation, then one
            # ones-outer-product replicates the row across all partitions
            row_all = smallp.tile([1, 512], fp32, tag="rowall")
            nc.scalar.activation(
                row_all[:], psum_row[:], func=mybir.ActivationFunctionType.Exp
            )
            psum_tab = psum_tp.tile([128, 512], fp32, tag="ptab")
            nc.tensor.matmul(
                psum_tab[:], ones_t[:], row_all[:], start=True, stop=True
            )
            table_b = gathp.tile([128, 512], fp32, tag=f"table{b}")
            nc.scalar.copy(table_b[:], psum_tab[:])
            tables.append(table_b)

        # ---- gathers: all four ICs back-to-back on the pool queue; each
        # IC's data is ready ~1.7us after dispatch, the ~28us post-drain
        # only blocks the next IC, and the last drain overlaps the tail ----
        gouts = []
        for b in range(BPC):
            gout = gathp.tile([128, 2048], fp32, tag=f"gout{b}")
            nc.gpsimd.ap_gather(
                gout[:],
                tables[b][:],
                idx_tiles[b][:],
                channels=128,
                num_elems=512,
                d=1,
                num_idxs=2048,
            )
            gouts.append(gout)

        for b in range(BPC):
            gout = gouts[b]
            # ---- compact: one partition per 16-group holds the real data ----
            C = smallp.tile([128, 128], fp32, tag=f"C{b}")
            gsel = gout[:].rearrange("(g s) k -> g s k", s=16)[:, 0, :]
            nc.sync.dma_start(C[:], gsel)

            # ---- windowed softmax normalize (4 segments x 32 / partition) --
            C3 = C[:].rearrange("p (s d) -> p s d", d=32)
            S = smallp.tile([128, 4], fp32, tag="S")
            nc.vector.tensor_reduce(
                out=S[:], in_=C3, axis=mybir.AxisListType.X, op=mybir.AluOpType.add
            )
            R = smallp.tile([128, 4], fp32, tag="R")
            nc.vector.reciprocal(R[:], S[:])
            O = smallp.tile([128, 128], fp32, tag="O")
            O3 = O[:].rearrange("p (s d) -> p s d", d=32)
            R3 = R[:].unsqueeze(2).broadcast_to((128, 4, 32))
            nc.vector.tensor_tensor(
                out=O3, in0=C3, in1=R3, op=mybir.AluOpType.mult
            )

            nc.sync.dma_start(out[b], O[:])

    nc.compile()
    return nc


def _prep_core_inputs(t2, idx_j, W2, v):
    import ml_dtypes

    g = (W2.T.astype(np.float64) @ v.astype(np.float64)).astype(np.float32)
    gbc = np.ascontiguousarray(
        np.broadcast_to(g.reshape(1, F2), (128, F2)).astype(ml_dtypes.bfloat16)
    )
    t2 = t2.astype(ml_dtypes.bfloat16)
    onesr = np.ones((1, 128), dtype=np.float32)
    ident = np.eye(128, dtype=np.float32)

    j3 = np.ascontiguousarray(idx_j.reshape(B, N1, DEG).astype(np.int16))
    in_maps = []
    for c in range(NCORES):
        bb = slice(BPC * c, BPC * (c + 1))
        t2s = np.ascontiguousarray(t2[bb])
        idxs = np.empty((BPC, 128, 128), dtype=np.int16)
        for lb in range(BPC):
            gb = BPC * c + lb
            for grp in range(8):
                stream = j3[gb, 64 * grp : 64 * (grp + 1), :].reshape(2048)
                idxs[lb, 16 * grp : 16 * (grp + 1), :] = stream.reshape(128, 16).T
        in_maps.append(
            {
                "t2s": t2s,
                "idxs": idxs,
                "gbc": gbc,
                "onesr": onesr,
                "ident": ident,
            }
        )
    return in_maps


def kernel(t1, t2, idx_b, idx_i, idx_j, W1, b1, W2, b2, v):
    from concourse.bass_utils import run_bass_kernel_spmd

    if "nc" not in _CACHE:
        _CACHE["nc"] = _build_program()
    nc = _CACHE["nc"]

    in_maps = _prep_core_inputs(
        np.asarray(t2, dtype=np.float32),
        np.asarray(idx_j),
        np.asarray(W2, dtype=np.float32),
        np.asarray(v, dtype=np.float32),
    )
    trace = bool(int(os.environ.get("KERNEL_TRACE", "0")))
    last_err = None
    for _attempt in range(3):
        try:
            res = run_bass_kernel_spmd(nc, in_maps, list(range(NCORES)), trace=trace)
            break
        except Exception as e:  # transient NRT_EXEC_UNIT_UNRECOVERABLE wedges
            last_err = e
    else:
        raise last_err
    _CACHE["last_results"] = res
    outs = [r["out"].reshape(BPC * N1 * DEG) for r in res.results]
    return np.concatenate(outs).astype(np.float32)



# revision 11
# speedup vs baseline: 2.7031x; 2.6933x over previous
"""Trainium2 Bass kernel for the sparse segment-softmax attention module.

Math: the reference computes, per nnz k,
    out[k] = segment_softmax((q1[b,i] + q2[b,j]) . v)  over segments (b, i).
Within a segment (fixed b, i), the q1[b,i].v term is constant and cancels in
softmax (shift invariance), as does the b2.v bias constant.  Hence
    out[k] = exp(u2[b, j_k]) / sum_{d in seg} exp(u2[b, j_d]),
    u2[b, n] = t2[b, n, :] . g,   g = W2^T v.

The baseline gathered exp(u2)[j] with the pool-engine IndirectCopy, which
is bound at ~28 ns per index per 16-partition group (~115 us for the 32768
gathers each NeuronCore owns).  ap_gather and dma_gather hit the same or
worse serial rates (measured).  This kernel instead does the gather on the
idle PE: the indices are kernel inputs, so the host uploads, per 128-nnz
block, a bf16 one-hot STATIONARY [128c x 128p] whose column p selects row
j%128; one LoadStationary+matmul against Ecol [128c x 4] (Ecol[c, hi] =
exp(u2[128*hi + c])) yields psum[p, 4*t+hi] = exp(u2[128*hi + L[p+128t]])
— 128 gathered candidate quads in ~130 PE cycles (~0.43 ns/value).  A DVE
multiply with a host-built hi-mask and a width-4 reduce selects the right
quadrant.  Per NeuronCore (2 batches): 256 stationary matmuls ~ 14 us on
the PE, overlapped with the one-hot upload stream.

Pipeline per core (2 of the 16 batches):
  - stream t2 shard (bf16); DVE mult by g-broadcast + ACT/DVE row-reduce
    -> u2acc [128, 4] (u2acc[p, t] = u2[128t + p]); ACT exp -> Ecol bf16.
  - 128 one-hot stationary matmuls per batch -> psum [128, 512].
  - DVE: psum * himask, reduce width-4 -> C [128, 128]; windowed segment
    softmax (sum 32, reciprocal, multiply); store.  Host applies the fixed
    inverse layout permutation.
"""

import os
from contextlib import ExitStack

import numpy as np

B = 16
N1 = 512
N2 = 512
F2 = 1024
DEG = 32
NNZ = B * N1 * DEG
NCORES = 8
BPC = B // NCORES  # batches per core
NBLK = 128  # one-hot blocks per batch (128 nnz each)

_CACHE: dict = {}


def _build_program():
    import concourse.bacc as bacc
    import concourse.mybir as mybir
    import concourse.tile as tile

    fp32 = mybir.dt.float32
    bf16 = mybir.dt.bfloat16

    nc = bacc.Bacc("TRN2", target_bir_lowering=False, debug=False)

    t2s = nc.dram_tensor("t2s", [BPC, N2, F2], bf16, kind="ExternalInput")
    gbc = nc.dram_tensor("gbc", [128, F2], bf16, kind="ExternalInput")
    # oh: one-hot stationaries, c-major: oh[b, c, 128t+p] = (J[p+128t] % 128 == c)
    oh = nc.dram_tensor("oh", [BPC, 128, NBLK * 128], bf16, kind="ExternalInput")
    # hm: hi-quadrant mask, hm[p, 4t+hi] = (J[p+128t] // 128 == hi)
    hm = nc.dram_tensor("hm", [BPC, 128, 4 * NBLK], bf16, kind="ExternalInput")
    out = nc.dram_tensor("out", [BPC, 128, 128], fp32, kind="ExternalOutput")

    with tile.TileContext(nc) as tc, ExitStack() as ctx:
        constp = ctx.enter_context(tc.tile_pool(name="const", bufs=1))
        t2p = ctx.enter_context(tc.tile_pool(name="t2p", bufs=4))
        prodp = ctx.enter_context(tc.tile_pool(name="prodp", bufs=3))
        ohp = ctx.enter_context(tc.tile_pool(name="ohp", bufs=1))
        smallp = ctx.enter_context(tc.tile_pool(name="small", bufs=2))
        psum_p = ctx.enter_context(tc.tile_pool(name="psg", bufs=2, space="PSUM"))

        # g first on the scalar ring (it gates the u2 mults); hi-masks after.
        g_sb = constp.tile([128, F2], bf16)
        nc.scalar.dma_start(g_sb[:], gbc[:])
        hm_tiles = []
        for b in range(BPC):
            hm_t = constp.tile([128, 4 * NBLK], bf16, tag=f"hm{b}", name=f"hm{b}")
            nc.scalar.dma_start(hm_t[:], hm[b])
            hm_tiles.append(hm_t)

        # t2 stream on the sync ring (2 MB), batch 0 first.
        t2_tiles = []
        for b in range(BPC):
            for t in range(4):
                t2t = t2p.tile([128, F2], bf16, tag="t2", name=f"t2t_{b}_{t}")
                nc.sync.dma_start(t2t[:], t2s[b, 128 * t : 128 * (t + 1), :])
                t2_tiles.append(t2t)

        # One-hot stationaries: one 4 MB partition-major DMA per batch on
        # the sync ring (streams behind the t2 tiles).
        oh_tiles = []
        for b in range(BPC):
            oht = ohp.tile([128, NBLK * 128], bf16, tag=f"oh{b}", name=f"oh{b}")
            nc.sync.dma_start(oht[:], oh[b])
            oh_tiles.append(oht)

        # u2 per batch: DVE mult + split ACT/DVE row-reduce (as baseline).
        ecols = []
        for b in range(BPC):
            u2accA = smallp.tile([128, 4], fp32, tag="u2accA")
            u2accB = smallp.tile([128, 4], fp32, tag="u2accB")
            u2acc = smallp.tile([128, 4], fp32, tag="u2acc")
            for t in range(4):
                t2t = t2_tiles[4 * b + t]
                prod = prodp.tile([128, F2], bf16, tag="prod")
                nc.vector.tensor_tensor(
                    out=prod[:], in0=t2t[:], in1=g_sb[:], op=mybir.AluOpType.mult
                )
                nc.scalar.activation(
                    prod[:, 0:512],
                    prod[:, 0:512],
                    func=mybir.ActivationFunctionType.Copy,
                    accum_out=u2accA[:, t : t + 1],
                )
                nc.vector.tensor_reduce(
                    out=u2accB[:, t : t + 1],
                    in_=prod[:, 512:1024],
                    axis=mybir.AxisListType.X,
                    op=mybir.AluOpType.add,
                )
                nc.vector.tensor_tensor(
                    out=u2acc[:, t : t + 1],
                    in0=u2accA[:, t : t + 1],
                    in1=u2accB[:, t : t + 1],
                    op=mybir.AluOpType.add,
                )
            ecol = smallp.tile([128, 4], bf16, tag=f"ecol{b}", name=f"ecol{b}")
            nc.scalar.activation(
                ecol[:], u2acc[:], func=mybir.ActivationFunctionType.Exp
            )
            ecols.append(ecol)

        # PE gather: one stationary matmul per 128-nnz block.
        psums = []
        for b in range(BPC):
            psum = psum_p.tile([128, 4 * NBLK], fp32, tag=f"ps{b}")
            for t in range(NBLK):
                nc.tensor.matmul(
                    psum[:, 4 * t : 4 * (t + 1)],
                    oh_tiles[b][:, 128 * t : 128 * (t + 1)],
                    ecols[b][:],
                    start=True,
                    stop=True,
                )
            psums.append(psum)

        # Quadrant select + windowed segment softmax + store.
        for b in range(BPC):
            sel = smallp.tile([128, 4 * NBLK], fp32, tag="sel")
            nc.vector.tensor_tensor(
                out=sel[:], in0=psums[b][:], in1=hm_tiles[b][:],
                op=mybir.AluOpType.mult,
            )
            c = smallp.tile([128, NBLK], fp32, tag="C")
            nc.vector.tensor_reduce(
                out=c[:],
                in_=sel[:].rearrange("p (t h) -> p t h", h=4),
                axis=mybir.AxisListType.X,
                op=mybir.AluOpType.add,
            )
            c3 = c[:].rearrange("p (q d) -> p q d", d=DEG)
            s = smallp.tile([128, 4], fp32, tag="S")
            nc.vector.tensor_reduce(
                out=s[:], in_=c3, axis=mybir.AxisListType.X, op=mybir.AluOpType.add
            )
            r = smallp.tile([128, 4], fp32, tag="R")
            nc.vector.reciprocal(r[:], s[:])
            o = smallp.tile([128, 128], fp32, tag="O")
            o3 = o[:].rearrange("p (q d) -> p q d", d=DEG)
            r3 = r[:].unsqueeze(2).broadcast_to((128, 4, DEG))
            nc.vector.tensor_tensor(out=o3, in0=c3, in1=r3, op=mybir.AluOpType.mult)
            nc.sync.dma_start(out[b], o[:])

    nc.compile()
    return nc


def _prep_core_inputs(t2, idx_j, W2, v):
    import ml_dtypes

    bf16 = ml_dtypes.bfloat16
    g = (W2.T.astype(np.float64) @ v.astype(np.float64)).astype(np.float32)
    gbc = np.ascontiguousarray(
        np.broadcast_to(g.reshape(1, F2), (128, F2)).astype(bf16)
    )
    t2 = t2.astype(bf16)

    # nnz (i, d) lands at C[p, t]: p = i % 128, t = 32*(i//128) + d
    i_arr = np.arange(N1)
    d_arr = np.arange(DEG)
    tt = (DEG * (i_arr[:, None] // 128) + d_arr[None, :])  # [512, 32]
    pp = np.broadcast_to((i_arr[:, None] % 128), (N1, DEG))

    j3 = np.asarray(idx_j).reshape(B, N1, DEG)
    in_maps = []
    eye = np.eye(128, dtype=bf16)
    hvals = np.arange(4, dtype=np.int32)
    for c in range(NCORES):
        bb = slice(BPC * c, BPC * (c + 1))
        ohs = np.empty((BPC, 128, NBLK * 128), dtype=bf16)
        hms = np.empty((BPC, 128, 4 * NBLK), dtype=bf16)
        for lb in range(BPC):
            gb = BPC * c + lb
            jmat = np.empty((128, NBLK), dtype=np.int32)  # jmat[p, t] = J
            jmat[pp.ravel(), tt.ravel()] = j3[gb].ravel()
            lo = jmat % 128
            hi = jmat // 128
            # ohs[lb][c_, 128t+p] = 1 iff c_ == lo[p, t]
            ohs[lb] = eye[:, lo.T].reshape(128, NBLK * 128)
            hms[lb] = (hi[:, :, None] == hvals).astype(bf16).reshape(128, 4 * NBLK)
        in_maps.append(
            {
                "t2s": np.ascontiguousarray(t2[bb]),
                "gbc": gbc,
                "oh": ohs,
                "hm": hms,
            }
        )
    return in_maps


def kernel(t1, t2, idx_b, idx_i, idx_j, W1, b1, W2, b2, v):
    from concourse.bass_utils import run_bass_kernel_spmd

    if "nc" not in _CACHE:
        _CACHE["nc"] = _build_program()
    nc = _CACHE["nc"]

    in_maps = _prep_core_inputs(
        np.asarray(t2, dtype=np.float32),
        np.asarray(idx_j),
        np.asarray(W2, dtype=np.float32),
        np.asarray(v, dtype=np.float32),
    )
    trace = bool(int(os.environ.get("KERNEL_TRACE", "0")))
    last_err = None
    for _attempt in range(3):
        try:
            res = run_bass_kernel_spmd(nc, in_maps, list(range(NCORES)), trace=trace)
            break
        except Exception as e:  # transient NRT_EXEC_UNIT_UNRECOVERABLE wedges
            last_err = e
    else:
        raise last_err
    _CACHE["last_results"] = res
    outs = []
    for r in res.results:
        o = r["out"].reshape(BPC, 128, 4, DEG)  # [b, p, q, d]
        o = o.transpose(0, 2, 1, 3).reshape(BPC * N1 * DEG)  # i = 128q + p
        outs.append(o)
    return np.concatenate(outs).astype(np.float32)
